# revision 12
# baseline (speedup 1.0000x reference)
"""Bidirectional LSTM (B=128, S=512, H=1024, V=128) for Trainium2, 8 NeuronCores.

Strategy
--------
The LSTM recurrence is irreducibly sequential (h_t depends on all of h_{t-1}
through a nonlinearity), and cross-core collectives have a ~20us latency floor
-- far above the ~16us per-step budget -- so each direction's recurrence runs
entirely on one core. The two directions are independent: cores 0-3 run the
forward pass, cores 4-7 the backward pass (identical SPMD program; direction
is encoded purely in the input data -- the backward cores just get the
sequence reversed). Redundant cores provide nothing but symmetry; the host
reads core 0 (fwd) and core 4 (bwd).

Per-step compute (the entire critical path) is one K=1152 matmul chain:
    gates[B,4H] = onehot_t[B,V] @ E_g  +  h_{t-1}[B,H] @ W_h
where E_g = emb @ W_x + b folds the embedding lookup AND bias into a single
one-hot matmul K-tile (VOCAB=128 = exactly one tile). All matmul operands are
float32r (TF32-like: 1 PE cycle/row vs 4 for fp32, ~1e-4 relative error).
Gate columns are reordered to (i,f,g,o) so chunk-by-chunk PSUM finalization
lets sigmoid/tanh/c-update overlap the remaining matmuls; h is transposed on
the PE (8x 128x128) into the stationary layout the next step needs. The
output projection logits^T += W_out_dir^T @ h^T is fused between steps
(batched over 2 steps so N=256 keeps fp32r at full rate); the two directions'
partial logits are summed on the host (a partial-sum unshard).
"""

import os
import numpy as np

B = 128
H = 1024
V = 128
S_FULL = 512
NK = 9            # K tiles: 1 one-hot + 8 h tiles
NH = H // 128     # 8
NCHUNK = 8        # 4096 gate cols / 512
SLOTS = 2         # steps per fused projection (N = SLOTS*128 = 256)

_NC_CACHE = {}
LAST_RUN_WALL = None
LAST_EXEC_NS = None


def _build_nc(S):
    from concourse import bacc
    import concourse.mybir as mybir
    import concourse.tile as tile

    f32 = mybir.dt.float32
    f32r = mybir.dt.float32r
    Sig = mybir.ActivationFunctionType.Sigmoid
    Tanh = mybir.ActivationFunctionType.Tanh

    nc = bacc.Bacc("TRN2", target_bir_lowering=False)

    wk = nc.dram_tensor("wk", [128, NK, 4096], f32, kind="ExternalInput")
    oh = nc.dram_tensor("oh", [S, 128, 128], mybir.dt.uint8,
                        kind="ExternalInput")
    h0t = nc.dram_tensor("h0t", [128, NH, 128], f32, kind="ExternalInput")
    c0 = nc.dram_tensor("c0", [128, H], f32, kind="ExternalInput")
    wout = nc.dram_tensor("wout", [128, NH, 128], f32, kind="ExternalInput")

    ptl = nc.dram_tensor("ptl", [S // SLOTS, 128, SLOTS * 128], f32,
                         kind="ExternalOutput")
    hN = nc.dram_tensor("hN", [128, H], f32, kind="ExternalOutput")
    cN = nc.dram_tensor("cN", [128, H], f32, kind="ExternalOutput")

    from concourse.masks import make_identity

    with tile.TileContext(nc) as tc:
        with tc.tile_pool(name="const", bufs=1) as const, \
             tc.tile_pool(name="ohp", bufs=2) as ohp, \
             tc.tile_pool(name="hist", bufs=2) as histp, \
             tc.tile_pool(name="cp", bufs=2) as cp, \
             tc.tile_pool(name="gate", bufs=1) as gatep, \
             tc.tile_pool(name="hp", bufs=1) as hp, \
             tc.tile_pool(name="pout", bufs=2) as poutp, \
             tc.tile_pool(name="gps", bufs=3, space="PSUM") as gpsp, \
             tc.tile_pool(name="tps", bufs=4, space="PSUM") as tpsp, \
             tc.tile_pool(name="pps", bufs=1, space="PSUM") as ppsp:

            # ---- one-time loads (gpsimd DMA casts f32 -> f32r = the rounding producer)
            wk_sb = const.tile([128, NK, 4096], f32r)
            for k in range(NK):
                nc.gpsimd.dma_start(wk_sb[:, k, :], wk[:, k, :])
            wout_sb = const.tile([128, NH, 128], f32r)
            nc.gpsimd.dma_start(wout_sb[:], wout[:])
            h0t_sb = const.tile([128, NH, 128], f32r)
            nc.gpsimd.dma_start(h0t_sb[:], h0t[:])

            ident_f32 = const.tile([128, 128], f32)
            make_identity(nc, ident_f32[:])
            ident = const.tile([128, 128], f32r)
            nc.vector.tensor_copy(ident[:], ident_f32[:])

            c_prev = cp.tile([128, H], f32)
            nc.sync.dma_start(c_prev[:], c0[:])

            hist_tiles = {}

            def lhsT(t, k):
                # stationary operand for h-contribution K-tile k at step t
                if t == 0:
                    return h0t_sb[:, k, :]
                return hist_tiles[(t - 1) // SLOTS][:, k, (t - 1) % SLOTS, :]

            for t in range(S):
                j, slot = t // SLOTS, t % SLOTS
                if slot == 0:
                    hist_t = histp.tile([128, NH, SLOTS, 128], f32r, tag="hist")
                    hist_tiles[j] = hist_t
                    hist_tiles.pop(j - 2, None)

                oh_t = ohp.tile([128, 128], f32r)
                nc.gpsimd.dma_start(oh_t[:], oh[t])

                # sio holds sigmoid(i), later overwritten by sigmoid(o);
                # tgc holds tanh(g), later overwritten by tanh(c) -- both
                # reuses are ordered by the data deps within the step.
                sio = gatep.tile([128, 1024], f32, tag="sio")
                sig_f = gatep.tile([128, 1024], f32, tag="sig_f")
                tgc = gatep.tile([128, 1024], f32, tag="tgc")
                h_t = hp.tile([128, H], f32r)
                c_new = cp.tile([128, H], f32)

                gates_sb = [sio, sio, sig_f, sig_f,
                            tgc, tgc, sio, sio]
                gate_fn = [Sig, Sig, Sig, Sig, Tanh, Tanh, Sig, Sig]

                for n in range(NCHUNK):
                    half = n % 2
                    hs = slice(half * 512, half * 512 + 512)
                    ps = gpsp.tile([128, 512], f32)
                    nc.tensor.matmul(ps[:], oh_t[:], wk_sb[:, 0, n * 512:(n + 1) * 512],
                                     start=True, stop=False)
                    for k in range(1, NK):
                        nc.tensor.matmul(ps[:], lhsT(t, k - 1),
                                         wk_sb[:, k, n * 512:(n + 1) * 512],
                                         start=False, stop=(k == NK - 1))
                    nc.scalar.activation(gates_sb[n][:, hs], ps[:], gate_fn[n])

                    if n in (4, 5):  # tanh_g half ready -> c update for this half
                        nc.vector.tensor_mul(tgc[:, hs], tgc[:, hs], sio[:, hs])
                        nc.vector.tensor_mul(c_new[:, hs], sig_f[:, hs], c_prev[:, hs])
                        nc.vector.tensor_add(c_new[:, hs], c_new[:, hs], tgc[:, hs])
                        nc.scalar.activation(tgc[:, hs], c_new[:, hs], Tanh)
                    if n in (6, 7):  # sig_o half ready -> h for this half
                        nc.vector.tensor_mul(h_t[:, hs], sio[:, hs], tgc[:, hs])

                def emit_transposes(kk_range):
                    for kk in kk_range:
                        tp = tpsp.tile([128, 128], f32r, tag="tp")
                        nc.tensor.matmul(tp[:], h_t[:, kk * 128:(kk + 1) * 128],
                                         ident[:], is_transpose=True,
                                         start=True, stop=True)
                        nc.vector.tensor_copy(hist_tiles[j][:, kk, slot, :], tp[:])

                def emit_proj(jp):
                    pp = ppsp.tile([128, SLOTS * 128], f32, tag="pp")
                    for k in range(NH):
                        nc.tensor.matmul(pp[:], wout_sb[:, k, :],
                                         hist_tiles[jp][:, k, :, :],
                                         start=(k == 0), stop=(k == NH - 1))
                    po = poutp.tile([128, SLOTS * 128], f32, tag="po")
                    nc.scalar.copy(po[:], pp[:])
                    nc.sync.dma_start(ptl[jp], po[:])

                # PE order: gate MMs -> T0..T3 (half-0 h long ready, no stall)
                # -> prior group's projection (covers half-1 eltwise latency)
                # -> T4..T7. Projection is delayed one group so its inputs are
                # never on the current step's critical chain.
                emit_transposes(range(0, 4))
                if slot == SLOTS - 1 and j >= 1:
                    emit_proj(j - 1)
                emit_transposes(range(4, 8))

                if t == S - 1:
                    nc.sync.dma_start(hN[:], h_t[:].bitcast(f32))
                    nc.sync.dma_start(cN[:], c_new[:])

                c_prev = c_new

            # projection of the final group (delayed-by-one scheme leaves it)
            jlast = S // SLOTS - 1
            pp = ppsp.tile([128, SLOTS * 128], f32, tag="pp")
            for k in range(NH):
                nc.tensor.matmul(pp[:], wout_sb[:, k, :],
                                 hist_tiles[jlast][:, k, :, :],
                                 start=(k == 0), stop=(k == NH - 1))
            po = poutp.tile([128, SLOTS * 128], f32, tag="po")
            nc.scalar.copy(po[:], pp[:])
            nc.sync.dma_start(ptl[jlast], po[:])

    nc.compile()
    return nc


def _dir_inputs(x_dir, h0, c0, W, b, W_out_dir, emb, S):
    """Per-direction device arrays. x_dir: [B, S] already in this direction's
    processing order (reversed for backward)."""
    f64 = np.float64
    W = np.asarray(W, f64)
    # reference gate column order (i, f, o, g) -> ours (i, f, g, o)
    perm = np.concatenate([np.arange(0, 2 * H), np.arange(3 * H, 4 * H),
                           np.arange(2 * H, 3 * H)])
    Wp = W[:, perm]
    bp = np.asarray(b, f64)[perm]
    E_g = np.asarray(emb, f64) @ Wp[:H] + bp          # [V, 4H], bias folded
    R = np.concatenate([E_g, Wp[H:]], 0)              # [V+H, 4H]
    wk = np.ascontiguousarray(
        R.reshape(NK, 128, 4096).transpose(1, 0, 2)).astype(np.float32)

    oh = np.zeros((S, V, B), np.uint8)
    t_idx = np.repeat(np.arange(S), B)
    b_idx = np.tile(np.arange(B), S)
    oh[t_idx, x_dir.T.ravel(), b_idx] = 1

    h0 = np.asarray(h0, np.float32)
    h0t = np.ascontiguousarray(
        h0.T.reshape(NH, 128, B).transpose(1, 0, 2)).astype(np.float32)
    wout = np.ascontiguousarray(
        np.asarray(W_out_dir, np.float32).reshape(NH, 128, V)
        .transpose(1, 0, 2)).astype(np.float32)

    return {"wk": wk, "oh": oh, "h0t": h0t,
            "c0": np.asarray(c0, np.float32), "wout": wout}


def kernel(x, h_f, h_b, c_f, c_b, emb, W_f, b_f, W_b, b_b, W_out, b_out):
    from concourse.bass_utils import run_bass_kernel_spmd

    x = np.asarray(x)
    S = x.shape[1]
    n_cores = int(os.environ.get("LSTM_CORES", "8"))

    if S not in _NC_CACHE:
        _NC_CACHE[S] = _build_nc(S)
    nc = _NC_CACHE[S]

    W_out = np.asarray(W_out, np.float32)
    in_f = _dir_inputs(x, h_f, c_f, W_f, b_f, W_out[:H], emb, S)
    in_b = _dir_inputs(x[:, ::-1], h_b, c_b, W_b, b_b, W_out[H:], emb, S)

    bwd_core = n_cores // 2
    in_maps = [in_f if c < bwd_core else in_b for c in range(n_cores)]
    import time as _time
    _t0 = _time.perf_counter()
    res = run_bass_kernel_spmd(nc, in_maps, core_ids=list(range(n_cores)),
                               trace=os.environ.get("LSTM_TRACE", "") == "1")
    global LAST_RUN_WALL, LAST_EXEC_NS
    LAST_RUN_WALL = _time.perf_counter() - _t0
    LAST_EXEC_NS = res.exec_time_ns

    rf, rb = res.results[0], res.results[bwd_core]
    # ptl [S//SLOTS, V, SLOTS*B] -> [V, S, B]
    Pf = rf["ptl"].reshape(S // SLOTS, V, SLOTS, B).transpose(1, 0, 2, 3) \
        .reshape(V, S, B)
    Pb = rb["ptl"].reshape(S // SLOTS, V, SLOTS, B).transpose(1, 0, 2, 3) \
        .reshape(V, S, B)[:, ::-1, :]
    logits = np.ascontiguousarray(
        (Pf + Pb).transpose(2, 1, 0)) + np.asarray(b_out, np.float32)

    return (logits.astype(np.float32), rf["hN"], rb["hN"], rf["cN"], rb["cN"])
